# revision 42
# baseline (speedup 1.0000x reference)
"""Trainium2 Bass kernel for nn_CrossModalAttention (GNN message passing).

8-core SPMD, edges sharded. Softmaxes computed without max subtraction
(scores bounded for this distribution, exp stays in f32 range), turning the
global edge softmax and per-node segment softmax into plain exp-sums.
Per-node sums accumulate on-chip via one-hot matmuls into a [72, N] SBUF
accumulator, AllReduced once across cores. Node k/q rows are fetched per
endpoint with GPSIMD dma_gather (int16 idx, 256B fp16 rows). The endpoint
stream (edge-attr rows per endpoint, grouped into 128-node windows) is
prepared host-side from edge_index. Bias folding: edge k-bias cancels in
the segment softmax; edge q-bias becomes a per-node factor folded into the
nv table; edge v-bias applied post-hoc as denom*bve.
"""

import numpy as np
from dataclasses import dataclass

import concourse.bass as bass
import concourse.bacc as bacc
import concourse.mybir as mybir
import concourse.tile as tile
from concourse.tile import add_dep_helper
from concourse import library_config
from concourse.masks import make_identity
from concourse.bass_utils import run_bass_kernel_spmd

F32 = mybir.dt.float32
F16 = mybir.dt.float16
I16 = mybir.dt.int16
I32 = mybir.dt.int32
AF = mybir.ActivationFunctionType
OP = mybir.AluOpType

ACC_C = 72  # accumulator rows: [0:4]=W1, [4:8]=denom2, [8:72]=aggv


@dataclass(frozen=True)
class Cfg:
    N: int = 20000
    E: int = 320000
    ND: int = 128
    ED: int = 64
    H: int = 4
    D: int = 16
    NCORE: int = 8
    GCALL: int = 32      # chunks per dma_gather call
    EABUF: int = 16      # chunks per ea cast-DMA buffer
    TBGRP: int = 8       # node tiles per staging group in table phase
    ESUB: int = 24       # edge row-tiles per edge-output sub-iteration
    W1BLK: int = 16      # node tiles per W1 stream block in pooled phase
    dbg: bool = False    # export intermediate tensors

    @property
    def HID(self):
        return self.H * self.D

    @property
    def SCALE(self):
        return float(self.D) ** -0.5

    @property
    def NT(self):
        return (self.N + 127) // 128

    @property
    def NPAD(self):
        return self.NT * 128

    @property
    def EC(self):
        return self.E // self.NCORE

    @property
    def ET(self):
        return (self.EC + 127) // 128

    @property
    def EPAD(self):
        return self.ET * 128


CFG_FULL = Cfg()


# --------------------------------------------------------------------------
# host-side prep (index-only metadata + sharding)
# --------------------------------------------------------------------------

def _prep(cfg, edge_index, edge_attr):
    row = np.asarray(edge_index[0], np.int64)
    col = np.asarray(edge_index[1], np.int64)
    EC, NT = cfg.EC, cfg.NT

    cores = []
    counts_all = np.zeros((cfg.NCORE, NT), np.int64)
    for c in range(cfg.NCORE):
        e0 = c * EC
        seg = np.concatenate([row[e0:e0 + EC], col[e0:e0 + EC]])
        eid = np.concatenate([np.arange(e0, e0 + EC), np.arange(e0, e0 + EC)])
        t_id = seg >> 7
        order = np.argsort(t_id, kind="stable")
        seg, eid, t_id = seg[order], eid[order], t_id[order]
        counts = np.bincount(t_id, minlength=NT)
        counts_all[c] = counts
        cores.append((seg, eid, counts))

    # common chunks-per-tile so one SPMD program fits every core
    kt = np.ceil(counts_all / 128.0).astype(np.int64).max(axis=0)
    kt = np.maximum(kt, 1)
    nchunk = int(kt.sum())
    nchunk += (-nchunk) % cfg.GCALL
    kt[-1] += nchunk - int(kt.sum())
    S = nchunk * 128

    tile_slot0 = np.zeros(NT, np.int64)
    tile_slot0[1:] = np.cumsum(kt * 128)[:-1]
    tile_of_slot = np.repeat(np.arange(NT), kt * 128)

    per_core = []
    for c in range(cfg.NCORE):
        seg, eid, counts = cores[c]
        csum = np.zeros(NT, np.int64)
        csum[1:] = np.cumsum(counts)[:-1]
        within = np.arange(len(seg)) - np.repeat(csum, counts)
        pos = np.repeat(tile_slot0, counts) + within

        gidx = np.zeros(S, np.int32)
        lid = np.full(S, -1.0, np.float16)
        ea_src = np.full(S, -1, np.int64)
        gidx[pos] = seg.astype(np.int32)
        lid[pos] = (seg - (tile_of_slot[pos] << 7)).astype(np.float16)
        ea_src[pos] = eid

        ea_rows = np.zeros((S, cfg.ED), np.float32)
        valid = ea_src >= 0
        ea_rows[valid] = edge_attr[ea_src[valid]]
        a = ea_rows.reshape(S // 256, 2, 128, cfg.ED).transpose(0, 1, 3, 2)
        a = a.reshape(S // 256, 2 * cfg.ED, 128)
        ea_pT = np.ascontiguousarray(
            a.transpose(1, 0, 2).reshape(2 * cfg.ED, (S // 256) * 128))

        lid_p = np.ascontiguousarray(lid.reshape(nchunk, 128).T)
        lid_rep = np.ascontiguousarray(np.broadcast_to(lid[None, :], (128, S)))

        per_core.append({"ea_pT": ea_pT, "lid_rep": lid_rep, "lid_p": lid_p})

    return {"kt": [int(v) for v in kt], "nchunk": nchunk}, per_core


def _pack_edge_shard(cfg, edge_attr, c):
    e0 = c * cfg.EC
    sh = np.zeros((cfg.EPAD, cfg.ED), np.float32)
    sh[:cfg.EC] = edge_attr[e0:e0 + cfg.EC]
    return np.ascontiguousarray(
        sh.reshape(cfg.ET, 128, cfg.ED).transpose(1, 0, 2))


# --------------------------------------------------------------------------
# device module
# --------------------------------------------------------------------------

def _build(cfg, meta):
    kt, nchunk = meta["kt"], meta["nchunk"]
    NT, NPAD, ET, H, D, ED, ND = (cfg.NT, cfg.NPAD, cfg.ET, cfg.H, cfg.D,
                                  cfg.ED, cfg.ND)
    HID = cfg.HID
    S = nchunk * 128
    KQ = 2 * HID
    W3 = 3 * HID

    nc = bacc.Bacc("TRN2", target_bir_lowering=False, debug=False,
                   num_devices=cfg.NCORE)

    x_in = nc.dram_tensor("x", [NPAD, ND], F32, kind="ExternalInput")
    ea_pT = nc.dram_tensor("ea_pT", [2 * ED, S // 2], F32, kind="ExternalInput")
    lidrep_in = nc.dram_tensor("lid_rep", [128, S], F16, kind="ExternalInput")
    lid_in = nc.dram_tensor("lid_p", [128, nchunk], F16, kind="ExternalInput")
    ea_sh = nc.dram_tensor("ea_shard", [128, ET, ED], F32, kind="ExternalInput")
    wn_in = nc.dram_tensor("Wn_cat", [ND, W3], F32, kind="ExternalInput")
    bn_in = nc.dram_tensor("bn_cat", [1, W3], F32, kind="ExternalInput")
    we_in = nc.dram_tensor("We_cat", [ED, W3], F32, kind="ExternalInput")
    won_in = nc.dram_tensor("Won", [HID, ND], F32, kind="ExternalInput")
    bon_in = nc.dram_tensor("bon", [1, ND], F32, kind="ExternalInput")
    woe_in = nc.dram_tensor("Woe", [HID, ED], F32, kind="ExternalInput")
    boe_in = nc.dram_tensor("boe", [1, ED], F32, kind="ExternalInput")
    bqe_in = nc.dram_tensor("bqe", [1, HID], F32, kind="ExternalInput")
    bve_in = nc.dram_tensor("bve_col", [HID, 1], F32, kind="ExternalInput")
    emat_in = nc.dram_tensor("E_mat", [H, HID], F32, kind="ExternalInput")
    mmat_in = nc.dram_tensor("M_mat", [HID, H], F32, kind="ExternalInput")
    out_x = nc.dram_tensor("out_x", [cfg.N, ND], F32, kind="ExternalOutput")
    out_e = nc.dram_tensor("out_e", [128, ET, ED], F32, kind="ExternalOutput")
    if cfg.dbg:
        dbg_tab = nc.dram_tensor("dbg_tab", [NPAD, KQ], F16,
                                 kind="ExternalOutput")
        dbg_nv = nc.dram_tensor("dbg_nv", [NPAD, HID], F16,
                                kind="ExternalOutput")
        dbg_acc = nc.dram_tensor("dbg_acc", [ACC_C, NPAD], F32,
                                 kind="ExternalOutput")
        dbg_bout = nc.dram_tensor("dbg_bout", [ACC_C, NPAD], F32,
                                  kind="ExternalOutput")
        dbg_gb = nc.dram_tensor("dbg_gb", [128, KQ], F16,
                                kind="ExternalOutput")
        dbg_eq = nc.dram_tensor("dbg_eq", [128, 4, W3], F32,
                                kind="ExternalOutput")
        dbg_ct = nc.dram_tensor("dbg_ct", [128, 4, 8 + HID], F16,
                                kind="ExternalOutput")
        dbg_oh = nc.dram_tensor("dbg_oh", [128, 4, 128], F16,
                                kind="ExternalOutput")

    with tile.TileContext(nc) as tc:
        with (
            tc.tile_pool(name="persist", bufs=1) as pp,
            tc.tile_pool(name="const", bufs=1) as cp,
            tc.tile_pool(name="dram", bufs=1, space="DRAM") as dram,
        ):
            nc.gpsimd.load_library(library_config.mlp)

            nkq_tab = dram.tile([NPAD, KQ], F16)
            nv_tab = dram.tile([NPAD, HID], F16)
            b_in = dram.tile([ACC_C, NPAD], F32)
            b_out = dram.tile(
                [ACC_C, NPAD], F32,
                addr_space="Shared" if cfg.NCORE > 4 else "Local")

            x_re = x_in[:].rearrange("(t p) d -> p t d", p=128)
            nkq_re = nkq_tab[:].rearrange("(t p) d -> p t d", p=128)
            nv_re = nv_tab[:].rearrange("(t p) d -> p t d", p=128)

            bexp = pp.tile([128, NT, H], F32)
            accum = pp.tile([ACC_C, NPAD], F32)
            lid_sb = pp.tile([128, nchunk], F16)

            # ---- constants ----
            ident = cp.tile([128, 128], F32)
            make_identity(nc, ident[:])
            ones_r = cp.tile([1, 128], F16)
            nc.vector.memset(ones_r[:], 1.0)
            ones_c = cp.tile([128, 1], F16)
            nc.vector.memset(ones_c[:], 1.0)
            ones_4 = cp.tile([H, 1], F16)
            nc.vector.memset(ones_4[:], 1.0)
            iota32 = cp.tile([128, 128], I32)
            nc.gpsimd.iota(iota32[:], [[1, 128]], base=0, channel_multiplier=0)
            iota4 = cp.tile([128, 4, 128], F16)
            for j in range(4):
                nc.vector.tensor_copy(iota4[:, j, :], iota32[:])
            iota_pc32 = cp.tile([128, 1], I32)
            nc.gpsimd.iota(iota_pc32[:], [[1, 1]], base=0, channel_multiplier=1)
            iota_pc = cp.tile([128, 1], F16)
            nc.vector.tensor_copy(iota_pc[:], iota_pc32[:])

            nc.vector.memset(accum[:], 0.0)
            nc.sync.dma_start(out=lid_sb[:], in_=lid_in[:])

            with (
                tc.tile_pool(name="wk0", bufs=2) as wk,
                tc.tile_pool(name="ps0", bufs=1, space="PSUM") as ps,
            ):
                def cast16(src_ap, shape, tag):
                    t32 = wk.tile(shape, F32, tag="cast32")
                    nc.sync.dma_start(out=t32[:], in_=src_ap)
                    t16 = cp.tile(shape, F16, tag=tag)
                    nc.vector.tensor_copy(t16[:], t32[:])
                    return t16

                wn16 = cast16(wn_in[:], [ND, W3], "wn16")
                we16 = cast16(we_in[:], [ED, W3], "we16")
                bn16 = cast16(bn_in[:], [1, W3], "bn16")
                won16 = cast16(won_in[:], [HID, ND], "won16")
                bon16 = cast16(bon_in[:], [1, ND], "bon16")
                woe16 = cast16(woe_in[:], [HID, ED], "woe16")
                emat16 = cast16(emat_in[:], [H, HID], "emat16")
                mmat16 = cast16(mmat_in[:], [HID, H], "mmat16")
                bqe16 = cast16(bqe_in[:], [1, HID], "bqe16")
                boe_sb = cp.tile([1, ED], F32)
                nc.sync.dma_start(out=boe_sb[:], in_=boe_in[:])
                bve_sb = cp.tile([HID, 1], F32)
                nc.sync.dma_start(out=bve_sb[:], in_=bve_in[:])
                we_dup = cp.tile([128, W3], F16, tag="wedup")
                nc.vector.tensor_copy(we_dup[0:ED, :], we16[:])
                nc.vector.tensor_copy(we_dup[ED:2 * ED, :], we16[:])
                bqe_ps = ps.tile([128, HID], F32, tag="bqeps")
                nc.tensor.matmul(bqe_ps[:], lhsT=ones_r[:], rhs=bqe16[:],
                                 start=True, stop=True)
                bqe_t = cp.tile([128, HID], F32)
                nc.vector.tensor_copy(bqe_t[:], bqe_ps[:])

            # ---- phase 1: node tables ----
            tab_writes = []
            with (
                tc.tile_pool(name="io1", bufs=2) as io,
                tc.tile_pool(name="wk1", bufs=3) as wk,
                tc.tile_pool(name="ps1", bufs=3, space="PSUM") as ps,
            ):
                for g0 in range(0, NT, cfg.TBGRP):
                    gn = min(cfg.TBGRP, NT - g0)
                    xs = io.tile([128, cfg.TBGRP, ND], F32, tag="xstage")
                    nc.sync.dma_start(out=xs[:, :gn, :],
                                      in_=x_re[:, g0:g0 + gn, :])
                    st = io.tile([128, cfg.TBGRP, W3], F16, tag="tstage")
                    for j in range(gn):
                        t = g0 + j
                        xT_ps = ps.tile([128, 128], F32, tag="xT")
                        nc.tensor.transpose(xT_ps[:], xs[:, j, :], ident[:])
                        xT16 = wk.tile([128, 128], F16, tag="xT16")
                        nc.vector.tensor_copy(xT16[:], xT_ps[:])
                        kqv_ps = ps.tile([128, W3], F32, tag="kqv")
                        nc.tensor.matmul(kqv_ps[:], lhsT=xT16[:], rhs=wn16[:],
                                         start=True, stop=False)
                        nc.tensor.matmul(kqv_ps[:], lhsT=ones_r[:], rhs=bn16[:],
                                         start=False, stop=True)
                        bt = wk.tile([128, HID], F32, tag="btmp")
                        nc.vector.tensor_tensor(bt[:], kqv_ps[:, 0:HID],
                                                bqe_t[:], op=OP.mult)
                        bs = wk.tile([128, H], F32, tag="bsum")
                        nc.vector.tensor_reduce(
                            bs[:], bt[:].rearrange("p (h d) -> p h d", h=H),
                            axis=mybir.AxisListType.X, op=OP.add)
                        nc.scalar.activation(bexp[:, t, :], bs[:], AF.Exp,
                                             scale=cfg.SCALE)
                        nc.vector.tensor_copy(st[:, j, 0:KQ], kqv_ps[:, 0:KQ])
                        nc.vector.tensor_tensor(
                            st[:, j, KQ:W3].rearrange("p (h d) -> p h d", h=H),
                            kqv_ps[:, KQ:W3].rearrange("p (h d) -> p h d", h=H),
                            bexp[:, t, :].to_broadcast([128, H, D]),
                            op=OP.mult)
                    tab_writes.append(nc.sync.dma_start(
                        out=nkq_re[:, g0:g0 + gn, :], in_=st[:, :gn, 0:KQ]))
                    tab_writes.append(nc.sync.dma_start(
                        out=nv_re[:, g0:g0 + gn, :], in_=st[:, :gn, KQ:W3]))

            # ---- phase 2: endpoint loop ----
            sched = []
            for t in range(NT):
                for j in range(kt[t]):
                    sched.append((t, j == 0, j == kt[t] - 1))
            assert len(sched) == nchunk

            with (
                tc.tile_pool(name="io2", bufs=2) as io,
                tc.tile_pool(name="wk2", bufs=3) as wk,
                tc.tile_pool(name="ps2w", bufs=2, space="PSUM") as psw,
                tc.tile_pool(name="ps2e", bufs=4, space="PSUM") as pse,
                tc.tile_pool(name="ps2x", bufs=2, space="PSUM") as psx,
            ):
                # fence: one op depending on all table writes; reads dep on it
                fence_t = wk.tile([1, 4], F32, tag="fence")
                fence = nc.vector.memset(fence_t[:], 0.0)
                for wi in tab_writes:
                    add_dep_helper(fence.ins, wi.ins,
                                   reason="fence after table writes")
                eabuf = win_ps = wbuf = lrbuf = None
                prod = contrib = onehot = None
                eq_group = []
                cur_tw = -1
                for k in range(nchunk):
                    t_k = sched[k][0]
                    if t_k // 8 != cur_tw:
                        cur_tw = t_k // 8
                        tn = min(8, NT - cur_tw * 8)
                        wbuf = io.tile([128, 8, KQ], F16, tag="wbuf")
                        w_inst = nc.sync.dma_start(
                            out=wbuf[:, :tn, :],
                            in_=nkq_re[:, cur_tw * 8:cur_tw * 8 + tn, :])
                        add_dep_helper(w_inst.ins, fence.ins,
                                       reason="window tile reads node table")
                    if k % 16 == 0:
                        h2 = k // 16
                        lw = min(16, nchunk - h2 * 16) * 128
                        lrbuf = io.tile([128, 16 * 128], F16, tag="lrbuf")
                        nc.sync.dma_start(
                            out=lrbuf[:, :lw],
                            in_=lidrep_in[:, h2 * 2048:h2 * 2048 + lw])
                    # transposed one-hot (window-node-major) -> PE expansion
                    onehotT = wk.tile([128, 128], F16, tag="onehotT")
                    nc.vector.tensor_tensor(
                        onehotT[:],
                        iota_pc[:].to_broadcast([128, 128]),
                        lrbuf[:, (k % 16) * 128:(k % 16 + 1) * 128],
                        op=OP.is_equal)
                    expkq_ps = psx.tile([128, KQ], F32, tag="expkq")
                    nc.tensor.matmul(expkq_ps[:], lhsT=onehotT[:],
                                     rhs=wbuf[:, t_k % 8, :],
                                     start=True, stop=True)
                    gbuf = wk.tile([128, KQ], F16, tag="gbuf")
                    nc.vector.tensor_copy(gbuf[:], expkq_ps[:])
                    if k % cfg.EABUF == 0:
                        h = k // cfg.EABUF
                        w = cfg.EABUF * 64
                        eabuf = io.tile([128, w], F16, tag="eabuf")
                        nc.gpsimd.dma_start(
                            out=eabuf[:], in_=ea_pT[:, h * w:(h + 1) * w])
                    if k % 4 == 0:
                        j4 = k // 4
                        prod = wk.tile([128, 4, KQ], F16, tag="prod")
                        contrib = wk.tile([128, 4, 8 + HID], F16, tag="contrib")
                        onehot = wk.tile([128, 4, 128], F16, tag="onehot")
                        eq_group = []
                        nc.vector.tensor_tensor(
                            onehot[:],
                            lid_sb[:, j4 * 4:(j4 + 1) * 4].to_broadcast(
                                [128, 4, 128]),
                            iota4[:], op=OP.is_equal)
                    if cfg.dbg and k == 0:
                        nc.sync.dma_start(out=dbg_gb[:], in_=gbuf[:])
                    q = k % 4
                    pr = (k // 2) % (cfg.EABUF // 2)
                    half = (k % 2) * ED
                    eqkv = pse.tile([128, W3], F32, tag="eqkv")
                    eq_group.append(eqkv)
                    nc.tensor.matmul(
                        eqkv[:],
                        lhsT=eabuf[half:half + ED, pr * 128:(pr + 1) * 128],
                        rhs=we_dup[half:half + ED, :], start=True, stop=True)
                    nc.vector.tensor_tensor(prod[:, q, :], eqkv[:, 0:KQ],
                                            gbuf[:], op=OP.mult)
                    if cfg.dbg and k < 4:
                        if k == 0:
                            dbg_eqs_t = pp.tile([128, 4, W3], F32,
                                                tag="dbgeqs")
                        nc.vector.tensor_copy(dbg_eqs_t[:, k, :], eqkv[:])
                        if k == 3:
                            nc.sync.dma_start(out=dbg_eq[:], in_=dbg_eqs_t[:])
                    if q == 3:
                        sraw = wk.tile([128, 4, 2 * H], F32, tag="sraw")
                        nc.vector.tensor_reduce(
                            sraw[:],
                            prod[:].rearrange("p c (g d) -> p c g d", d=D),
                            axis=mybir.AxisListType.X, op=OP.add)
                        nc.scalar.activation(contrib[:, :, 0:2 * H], sraw[:],
                                             AF.Exp, scale=cfg.SCALE)
                        for q2 in range(4):
                            k2 = k - 3 + q2
                            eq2 = eq_group[q2]
                            nc.vector.tensor_tensor(
                                contrib[:, q2, 8:].rearrange(
                                    "p (h d) -> p h d", h=H),
                                contrib[:, q2, H:2 * H].to_broadcast(
                                    [128, H, D]),
                                eq2[:, KQ:W3].rearrange("p (h d) -> p h d",
                                                        h=H),
                                op=OP.mult)
                            t2, wf2, wl2 = sched[k2]
                            if wf2:
                                win_ps = psw.tile([ACC_C, 128], F32, tag="win")
                            nc.tensor.matmul(win_ps[:],
                                             lhsT=contrib[:, q2, :],
                                             rhs=onehot[:, q2, :],
                                             start=wf2, stop=wl2)
                            if wl2:
                                nc.vector.tensor_copy(
                                    accum[:, t2 * 128:(t2 + 1) * 128],
                                    win_ps[:])
                        if cfg.dbg and k == 3:
                            nc.sync.dma_start(out=dbg_ct[:], in_=contrib[:])
                            nc.sync.dma_start(out=dbg_oh[:], in_=onehot[:])

            if cfg.dbg:
                nc.sync.dma_start(out=dbg_acc[:], in_=accum[:])
                nc.sync.dma_start(out=dbg_tab[:], in_=nkq_tab[:])
                nc.sync.dma_start(out=dbg_nv[:], in_=nv_tab[:])

            # ---- phase 3: AllReduce ----
            nc.gpsimd.dma_start(b_in[:], accum[:])
            nc.gpsimd.collective_compute(
                "AllReduce", OP.add,
                replica_groups=[list(range(cfg.NCORE))],
                ins=[b_in.opt()], outs=[b_out.opt()])

            if cfg.dbg:
                dbg_bt_sb = pp.tile([ACC_C, NPAD], F32, tag="dbgbt")
                nc.sync.dma_start(out=dbg_bt_sb[:], in_=b_out[:])
                nc.sync.dma_start(out=dbg_bout[:], in_=dbg_bt_sb[:])

            # ---- phase 4a: pooled vector -> edge row ----
            with (
                tc.tile_pool(name="io4", bufs=2) as io,
                tc.tile_pool(name="wk4", bufs=2) as wk,
                tc.tile_pool(name="ps4", bufs=1, space="PSUM") as ps,
                tc.tile_pool(name="ps4w", bufs=1, space="PSUM") as psw,
                tc.tile_pool(name="ps4t", bufs=2, space="PSUM") as pst,
            ):
                nv_all = io.tile([128, NT, HID + 1], F16, tag="nvall", bufs=1)
                nc.vector.memset(nv_all[:], 1.0)
                nv_rd = nc.sync.dma_start(out=nv_all[:, :, 0:HID],
                                          in_=nv_re[:, :, :])
                for wi in tab_writes:
                    add_dep_helper(nv_rd.ins, wi.ins,
                                   reason="nv readback after table write")
                pv_ps = ps.tile([HID + 1, H], F32, tag="pv")
                nblk = cfg.W1BLK
                for b0 in range(0, NT, nblk):
                    bn_ = min(nblk, NT - b0)
                    w1blk = io.tile([H, nblk * 128], F32, tag="w1blk")
                    nc.sync.dma_start(
                        out=w1blk[:, :bn_ * 128],
                        in_=b_out[0:H, b0 * 128:(b0 + bn_) * 128])
                    for j in range(bn_):
                        t = b0 + j
                        w1T_ps = pst.tile([128, H], F32, tag="w1T")
                        nc.tensor.transpose(
                            w1T_ps[:], w1blk[:, j * 128:(j + 1) * 128],
                            ident[0:H, 0:H])
                        w1t16 = wk.tile([128, H], F16, tag="w1t16")
                        nc.vector.tensor_copy(w1t16[:], w1T_ps[:])
                        nc.tensor.matmul(pv_ps[:, :],
                                         lhsT=nv_all[:, t, :],
                                         rhs=w1t16[:], start=(t == 0),
                                         stop=(t == NT - 1))
                t1_16 = wk.tile([HID, H], F16, tag="t1")
                nc.vector.tensor_tensor(t1_16[:], pv_ps[0:HID, :], mmat16[:],
                                        op=OP.mult)
                den1_sb = wk.tile([1, H], F32, tag="den1sb")
                nc.vector.tensor_copy(den1_sb[:], pv_ps[HID:HID + 1, 0:H])
                den1T_ps = psw.tile([H, 1], F32, tag="den1T")
                nc.tensor.transpose(den1T_ps[:], den1_sb[:], ident[0:1, 0:1])
                den1r = wk.tile([H, 1], F32, tag="den1r")
                nc.vector.reciprocal(den1r[:], den1T_ps[:])
                wpv_ps = psw.tile([H, ED], F32, tag="wpv")
                nc.tensor.matmul(wpv_ps[:], lhsT=t1_16[:], rhs=woe16[:],
                                 start=True, stop=True)
                scl16 = wk.tile([H, ED], F16, tag="scl")
                nc.vector.tensor_scalar(scl16[:], wpv_ps[:], den1r[:], None,
                                        op0=OP.mult)
                row_ps = psw.tile([1, ED], F32, tag="rowps")
                nc.tensor.matmul(row_ps[:], lhsT=ones_4[:], rhs=scl16[:],
                                 start=True, stop=True)
                row16 = wk.tile([1, ED], F16, tag="row16")
                nc.vector.tensor_tensor(row16[:], row_ps[:], boe_sb[:],
                                        op=OP.add)
                rowb_ps = psw.tile([128, ED], F32, tag="rowb")
                nc.tensor.matmul(rowb_ps[:], lhsT=ones_r[:], rhs=row16[:],
                                 start=True, stop=True)
                rowb = cp.tile([128, 1, ED], F32)
                nc.vector.tensor_copy(rowb[:, 0, :], rowb_ps[:])

            # ---- phase 4b: edge output ----
            with (
                tc.tile_pool(name="io5", bufs=3) as io,
            ):
                for s0 in range(0, ET, cfg.ESUB):
                    sn = min(cfg.ESUB, ET - s0)
                    et = io.tile([128, cfg.ESUB, ED], F32, tag="etile")
                    nc.sync.dma_start(out=et[:, :sn, :],
                                      in_=ea_sh[:, s0:s0 + sn, :])
                    nc.vector.tensor_tensor(
                        et[:, :sn, :], et[:, :sn, :],
                        rowb[:].to_broadcast([128, sn, ED]), op=OP.add)
                    nc.sync.dma_start(out=out_e[:, s0:s0 + sn, :],
                                      in_=et[:, :sn, :])

            # ---- phase 4c: node output (redundant on every core) ----
            with (
                tc.tile_pool(name="wk6", bufs=3) as wk,
                tc.tile_pool(name="ps6", bufs=2, space="PSUM") as ps,
            ):
                for t in range(NT):
                    base = t * 128
                    rows = min(128, cfg.N - base)
                    if rows <= 0:
                        break
                    den2 = wk.tile([H, 128], F32, tag="den2")
                    aggv = wk.tile([HID, 128], F32, tag="aggv")
                    nc.sync.dma_start(out=den2[:, :rows],
                                      in_=b_out[H:2 * H, base:base + rows])
                    nc.sync.dma_start(out=aggv[:, :rows],
                                      in_=b_out[2 * H:ACC_C, base:base + rows])
                    den2_16 = wk.tile([H, 128], F16, tag="den216")
                    nc.vector.tensor_copy(den2_16[:, :rows], den2[:, :rows])
                    dx_ps = ps.tile([HID, 128], F32, tag="dxps")
                    nc.tensor.matmul(dx_ps[:, :rows], lhsT=emat16[:],
                                     rhs=den2_16[:, :rows], start=True,
                                     stop=True)
                    at = wk.tile([HID, 128], F32, tag="at")
                    nc.vector.scalar_tensor_tensor(
                        at[:, :rows], dx_ps[:, :rows], bve_sb[:],
                        aggv[:, :rows], op0=OP.mult, op1=OP.add)
                    dcl = wk.tile([HID, 128], F32, tag="dcl")
                    nc.vector.tensor_scalar(dcl[:, :rows], dx_ps[:, :rows],
                                            1e-20, None, op0=OP.max)
                    drec = wk.tile([HID, 128], F32, tag="drec")
                    nc.vector.reciprocal(drec[:, :rows], dcl[:, :rows])
                    agg16 = wk.tile([HID, 128], F16, tag="agg16")
                    nc.vector.tensor_tensor(agg16[:, :rows], at[:, :rows],
                                            drec[:, :rows], op=OP.mult)
                    o_ps = ps.tile([128, ND], F32, tag="ops")
                    nc.tensor.matmul(o_ps[:rows, :], lhsT=agg16[:, :rows],
                                     rhs=won16[:], start=True, stop=False)
                    nc.tensor.matmul(o_ps[:rows, :], lhsT=ones_r[:, :rows],
                                     rhs=bon16[:], start=False, stop=True)
                    xres = wk.tile([128, ND], F32, tag="xres")
                    nc.sync.dma_start(out=xres[:rows, :],
                                      in_=x_in[base:base + rows, :])
                    ot = wk.tile([128, ND], F32, tag="ot")
                    nc.vector.tensor_add(ot[:rows, :], o_ps[:rows, :],
                                         xres[:rows, :])
                    nc.sync.dma_start(out=out_x[base:base + rows, :],
                                      in_=ot[:rows, :])

    nc.compile()
    return nc


# --------------------------------------------------------------------------
# entry point
# --------------------------------------------------------------------------

_CACHE = {}


def run(cfg, inputs, run_fn=None):
    x = np.asarray(inputs["x"], np.float32)
    edge_attr = np.asarray(inputs["edge_attr"], np.float32)
    edge_index = np.asarray(inputs["edge_index"])

    meta, per_core = _prep(cfg, edge_index, edge_attr)
    key = (cfg, tuple(meta["kt"]), meta["nchunk"])
    if key not in _CACHE:
        _CACHE[key] = _build(cfg, meta)
    nc = _CACHE[key]

    x_pad = np.zeros((cfg.NPAD, cfg.ND), np.float32)
    x_pad[:cfg.N] = x
    H, D, HID = cfg.H, cfg.D, cfg.HID
    wn_cat = np.concatenate(
        [inputs["Wkn"], inputs["Wqn"], inputs["Wvn"]], axis=1)
    bn_cat = np.concatenate(
        [inputs["bkn"], inputs["bqn"], inputs["bvn"]])[None, :]
    we_cat = np.concatenate(
        [inputs["Wqe"], inputs["Wke"], inputs["Wve"]], axis=1)
    emat = np.zeros((H, HID), np.float32)
    for h in range(H):
        emat[h, h * D:(h + 1) * D] = 1.0
    shared = {
        "x": x_pad,
        "Wn_cat": np.ascontiguousarray(wn_cat, dtype=np.float32),
        "bn_cat": np.ascontiguousarray(bn_cat, dtype=np.float32),
        "We_cat": np.ascontiguousarray(we_cat, dtype=np.float32),
        "Won": np.asarray(inputs["Won"], np.float32),
        "bon": np.asarray(inputs["bon"], np.float32)[None, :].copy(),
        "Woe": np.asarray(inputs["Woe"], np.float32),
        "boe": np.asarray(inputs["boe"], np.float32)[None, :].copy(),
        "bqe": np.asarray(inputs["bqe"], np.float32)[None, :].copy(),
        "bve_col": np.asarray(inputs["bve"], np.float32)[:, None].copy(),
        "E_mat": emat, "M_mat": np.ascontiguousarray(emat.T),
    }
    in_maps = []
    for c in range(cfg.NCORE):
        m = dict(shared)
        m.update(per_core[c])
        m["ea_shard"] = _pack_edge_shard(cfg, edge_attr, c)
        in_maps.append(m)

    if run_fn is not None:
        results = run_fn(nc, in_maps)
    else:
        res = run_bass_kernel_spmd(nc, in_maps,
                                   core_ids=list(range(cfg.NCORE)))
        results = res.results

    upd_x = np.asarray(results[0]["out_x"], np.float32)
    upd_e = np.zeros((cfg.E, cfg.ED), np.float32)
    for c in range(cfg.NCORE):
        pe = results[c]["out_e"].transpose(1, 0, 2).reshape(cfg.EPAD, cfg.ED)
        upd_e[c * cfg.EC:(c + 1) * cfg.EC] = pe[:cfg.EC]
    return upd_x, upd_e


def kernel(**inputs):
    return run(CFG_FULL, inputs)


# revision 43
# speedup vs baseline: 1.7217x; 1.7217x over previous
"""Trainium2 Bass kernel for nn_CrossModalAttention (GNN message passing).

8-core SPMD, edges sharded. Softmaxes computed without max subtraction
(scores bounded for this distribution, exp stays in f32 range), turning the
global edge softmax and per-node segment softmax into plain exp-sums.
Per-node sums accumulate on-chip via one-hot matmuls into a [72, N] SBUF
accumulator, AllReduced once across cores. Node k/q rows are fetched per
endpoint with GPSIMD dma_gather (int16 idx, 256B fp16 rows). The endpoint
stream (edge-attr rows per endpoint, grouped into 128-node windows) is
prepared host-side from edge_index. Bias folding: edge k-bias cancels in
the segment softmax; edge q-bias becomes a per-node factor folded into the
nv table; edge v-bias applied post-hoc as denom*bve.
"""

import numpy as np
from dataclasses import dataclass

import concourse.bass as bass
import concourse.bacc as bacc
import concourse.mybir as mybir
import concourse.tile as tile
from concourse.tile import add_dep_helper
from concourse import library_config
from concourse.masks import make_identity
from concourse.bass_utils import run_bass_kernel_spmd

F32 = mybir.dt.float32
F16 = mybir.dt.float16
I16 = mybir.dt.int16
I32 = mybir.dt.int32
AF = mybir.ActivationFunctionType
OP = mybir.AluOpType

ACC_C = 72  # accumulator rows: [0:4]=W1, [4:8]=denom2, [8:72]=aggv


@dataclass(frozen=True)
class Cfg:
    N: int = 20000
    E: int = 320000
    ND: int = 128
    ED: int = 64
    H: int = 4
    D: int = 16
    NCORE: int = 8
    GCALL: int = 32      # chunks per dma_gather call
    EABUF: int = 16      # chunks per ea cast-DMA buffer
    TBGRP: int = 8       # node tiles per staging group in table phase
    ESUB: int = 24       # edge row-tiles per edge-output sub-iteration
    W1BLK: int = 16      # node tiles per W1 stream block in pooled phase
    dbg: bool = False    # export intermediate tensors

    @property
    def HID(self):
        return self.H * self.D

    @property
    def SCALE(self):
        return float(self.D) ** -0.5

    @property
    def NT(self):
        return (self.N + 127) // 128

    @property
    def NPAD(self):
        return self.NT * 128

    @property
    def EC(self):
        return self.E // self.NCORE

    @property
    def ET(self):
        return (self.EC + 127) // 128

    @property
    def EPAD(self):
        return self.ET * 128


CFG_FULL = Cfg()


# --------------------------------------------------------------------------
# host-side prep (index-only metadata + sharding)
# --------------------------------------------------------------------------

def _prep(cfg, edge_index, edge_attr):
    row = np.asarray(edge_index[0], np.int64)
    col = np.asarray(edge_index[1], np.int64)
    EC, NT = cfg.EC, cfg.NT

    cores = []
    counts_all = np.zeros((cfg.NCORE, NT), np.int64)
    for c in range(cfg.NCORE):
        e0 = c * EC
        seg = np.concatenate([row[e0:e0 + EC], col[e0:e0 + EC]])
        eid = np.concatenate([np.arange(e0, e0 + EC), np.arange(e0, e0 + EC)])
        t_id = seg >> 7
        order = np.argsort(t_id, kind="stable")
        seg, eid, t_id = seg[order], eid[order], t_id[order]
        counts = np.bincount(t_id, minlength=NT)
        counts_all[c] = counts
        cores.append((seg, eid, counts))

    # common chunks-per-tile so one SPMD program fits every core
    kt = np.ceil(counts_all / 128.0).astype(np.int64).max(axis=0)
    kt = np.maximum(kt, 1)
    nchunk = int(kt.sum())
    nchunk += (-nchunk) % cfg.GCALL
    kt[-1] += nchunk - int(kt.sum())
    S = nchunk * 128

    tile_slot0 = np.zeros(NT, np.int64)
    tile_slot0[1:] = np.cumsum(kt * 128)[:-1]
    tile_of_slot = np.repeat(np.arange(NT), kt * 128)

    per_core = []
    for c in range(cfg.NCORE):
        seg, eid, counts = cores[c]
        csum = np.zeros(NT, np.int64)
        csum[1:] = np.cumsum(counts)[:-1]
        within = np.arange(len(seg)) - np.repeat(csum, counts)
        pos = np.repeat(tile_slot0, counts) + within

        gidx = np.zeros(S, np.int32)
        lid = np.full(S, -1.0, np.float16)
        ea_src = np.full(S, -1, np.int64)
        gidx[pos] = seg.astype(np.int32)
        lid[pos] = (seg - (tile_of_slot[pos] << 7)).astype(np.float16)
        ea_src[pos] = eid

        ea_rows = np.zeros((S, cfg.ED), np.float32)
        valid = ea_src >= 0
        ea_rows[valid] = edge_attr[ea_src[valid]]
        a = ea_rows.reshape(S // 256, 2, 128, cfg.ED).transpose(0, 1, 3, 2)
        a = a.reshape(S // 256, 2 * cfg.ED, 128)
        ea_pT = np.ascontiguousarray(
            a.transpose(1, 0, 2).reshape(2 * cfg.ED, (S // 256) * 128))

        lid_p = np.ascontiguousarray(lid.reshape(nchunk, 128).T)
        lid_rep = np.ascontiguousarray(np.broadcast_to(lid[None, :], (128, S)))

        per_core.append({"ea_pT": ea_pT, "lid_rep": lid_rep, "lid_p": lid_p})

    return {"kt": [int(v) for v in kt], "nchunk": nchunk}, per_core


def _pack_edge_shard(cfg, edge_attr, c):
    e0 = c * cfg.EC
    sh = np.zeros((cfg.EPAD, cfg.ED), np.float32)
    sh[:cfg.EC] = edge_attr[e0:e0 + cfg.EC]
    return np.ascontiguousarray(
        sh.reshape(cfg.ET, 128, cfg.ED).transpose(1, 0, 2))


# --------------------------------------------------------------------------
# device module
# --------------------------------------------------------------------------

def _build(cfg, meta):
    kt, nchunk = meta["kt"], meta["nchunk"]
    NT, NPAD, ET, H, D, ED, ND = (cfg.NT, cfg.NPAD, cfg.ET, cfg.H, cfg.D,
                                  cfg.ED, cfg.ND)
    HID = cfg.HID
    S = nchunk * 128
    KQ = 2 * HID
    W3 = 3 * HID

    nc = bacc.Bacc("TRN2", target_bir_lowering=False, debug=False,
                   num_devices=cfg.NCORE)

    x_in = nc.dram_tensor("x", [NPAD, ND], F32, kind="ExternalInput")
    ea_pT = nc.dram_tensor("ea_pT", [2 * ED, S // 2], F32, kind="ExternalInput")
    lidrep_in = nc.dram_tensor("lid_rep", [128, S], F16, kind="ExternalInput")
    lid_in = nc.dram_tensor("lid_p", [128, nchunk], F16, kind="ExternalInput")
    ea_sh = nc.dram_tensor("ea_shard", [128, ET, ED], F32, kind="ExternalInput")
    wn_in = nc.dram_tensor("Wn_cat", [ND, W3], F32, kind="ExternalInput")
    bn_in = nc.dram_tensor("bn_cat", [1, W3], F32, kind="ExternalInput")
    we_in = nc.dram_tensor("We_cat", [ED, W3], F32, kind="ExternalInput")
    won_in = nc.dram_tensor("Won", [HID, ND], F32, kind="ExternalInput")
    bon_in = nc.dram_tensor("bon", [1, ND], F32, kind="ExternalInput")
    woe_in = nc.dram_tensor("Woe", [HID, ED], F32, kind="ExternalInput")
    boe_in = nc.dram_tensor("boe", [1, ED], F32, kind="ExternalInput")
    bqe_in = nc.dram_tensor("bqe", [1, HID], F32, kind="ExternalInput")
    bve_in = nc.dram_tensor("bve_col", [HID, 1], F32, kind="ExternalInput")
    emat_in = nc.dram_tensor("E_mat", [H, HID], F32, kind="ExternalInput")
    mmat_in = nc.dram_tensor("M_mat", [HID, H], F32, kind="ExternalInput")
    out_x = nc.dram_tensor("out_x", [cfg.N, ND], F32, kind="ExternalOutput")
    out_e = nc.dram_tensor("out_e", [128, ET, ED], F32, kind="ExternalOutput")
    if cfg.dbg:
        dbg_tab = nc.dram_tensor("dbg_tab", [NPAD, KQ], F16,
                                 kind="ExternalOutput")
        dbg_nv = nc.dram_tensor("dbg_nv", [NPAD, HID], F16,
                                kind="ExternalOutput")
        dbg_acc = nc.dram_tensor("dbg_acc", [ACC_C, NPAD], F32,
                                 kind="ExternalOutput")
        dbg_bout = nc.dram_tensor("dbg_bout", [ACC_C, NPAD], F32,
                                  kind="ExternalOutput")
        dbg_gb = nc.dram_tensor("dbg_gb", [128, KQ], F16,
                                kind="ExternalOutput")
        dbg_eq = nc.dram_tensor("dbg_eq", [128, 4, W3], F32,
                                kind="ExternalOutput")
        dbg_ct = nc.dram_tensor("dbg_ct", [128, 4, 8 + HID], F16,
                                kind="ExternalOutput")
        dbg_oh = nc.dram_tensor("dbg_oh", [128, 4, 128], F16,
                                kind="ExternalOutput")

    with tile.TileContext(nc) as tc:
        with (
            tc.tile_pool(name="persist", bufs=1) as pp,
            tc.tile_pool(name="const", bufs=1) as cp,
            tc.tile_pool(name="dram", bufs=1, space="DRAM") as dram,
        ):
            nc.gpsimd.load_library(library_config.mlp)

            nkq_tab = dram.tile([NPAD, KQ], F16)
            nv_tab = dram.tile([NPAD, HID], F16)
            b_in = dram.tile([ACC_C, NPAD], F32)
            b_out = dram.tile(
                [ACC_C, NPAD], F32,
                addr_space="Shared" if cfg.NCORE > 4 else "Local")

            x_re = x_in[:].rearrange("(t p) d -> p t d", p=128)
            nkq_re = nkq_tab[:].rearrange("(t p) d -> p t d", p=128)
            nv_re = nv_tab[:].rearrange("(t p) d -> p t d", p=128)

            bexp = pp.tile([128, NT, H], F32)
            accum = pp.tile([ACC_C, NPAD], F32)
            lid_sb = pp.tile([128, nchunk], F16)

            # ---- constants ----
            ident = cp.tile([128, 128], F32)
            make_identity(nc, ident[:])
            ones_r = cp.tile([1, 128], F16)
            nc.vector.memset(ones_r[:], 1.0)
            ones_c = cp.tile([128, 1], F16)
            nc.vector.memset(ones_c[:], 1.0)
            ones_4 = cp.tile([H, 1], F16)
            nc.vector.memset(ones_4[:], 1.0)
            iota32 = cp.tile([128, 128], I32)
            nc.gpsimd.iota(iota32[:], [[1, 128]], base=0, channel_multiplier=0)
            iota4 = cp.tile([128, 4, 128], F16)
            for j in range(4):
                nc.vector.tensor_copy(iota4[:, j, :], iota32[:])
            iota_pc32 = cp.tile([128, 1], I32)
            nc.gpsimd.iota(iota_pc32[:], [[1, 1]], base=0, channel_multiplier=1)
            iota_pc = cp.tile([128, 1], F16)
            nc.vector.tensor_copy(iota_pc[:], iota_pc32[:])

            nc.vector.memset(accum[:], 0.0)
            nc.sync.dma_start(out=lid_sb[:], in_=lid_in[:])

            with (
                tc.tile_pool(name="wk0", bufs=2) as wk,
                tc.tile_pool(name="ps0", bufs=1, space="PSUM") as ps,
            ):
                def cast16(src_ap, shape, tag):
                    t32 = wk.tile(shape, F32, tag="cast32")
                    nc.sync.dma_start(out=t32[:], in_=src_ap)
                    t16 = cp.tile(shape, F16, tag=tag)
                    nc.vector.tensor_copy(t16[:], t32[:])
                    return t16

                wn16 = cast16(wn_in[:], [ND, W3], "wn16")
                we16 = cast16(we_in[:], [ED, W3], "we16")
                bn16 = cast16(bn_in[:], [1, W3], "bn16")
                won16 = cast16(won_in[:], [HID, ND], "won16")
                bon16 = cast16(bon_in[:], [1, ND], "bon16")
                woe16 = cast16(woe_in[:], [HID, ED], "woe16")
                emat16 = cast16(emat_in[:], [H, HID], "emat16")
                mmat16 = cast16(mmat_in[:], [HID, H], "mmat16")
                bqe16 = cast16(bqe_in[:], [1, HID], "bqe16")
                boe_sb = cp.tile([1, ED], F32)
                nc.sync.dma_start(out=boe_sb[:], in_=boe_in[:])
                bve_sb = cp.tile([HID, 1], F32)
                nc.sync.dma_start(out=bve_sb[:], in_=bve_in[:])
                we_dup = cp.tile([128, W3], F16, tag="wedup")
                nc.vector.tensor_copy(we_dup[0:ED, :], we16[:])
                nc.vector.tensor_copy(we_dup[ED:2 * ED, :], we16[:])
                bqe_ps = ps.tile([128, HID], F32, tag="bqeps")
                nc.tensor.matmul(bqe_ps[:], lhsT=ones_r[:], rhs=bqe16[:],
                                 start=True, stop=True)
                bqe_t = cp.tile([128, HID], F32)
                nc.vector.tensor_copy(bqe_t[:], bqe_ps[:])

            # ---- phase 1: node tables ----
            tab_writes = []
            with (
                tc.tile_pool(name="io1", bufs=2) as io,
                tc.tile_pool(name="wk1", bufs=3) as wk,
                tc.tile_pool(name="ps1", bufs=3, space="PSUM") as ps,
            ):
                for g0 in range(0, NT, cfg.TBGRP):
                    gn = min(cfg.TBGRP, NT - g0)
                    xs = io.tile([128, cfg.TBGRP, ND], F32, tag="xstage")
                    nc.sync.dma_start(out=xs[:, :gn, :],
                                      in_=x_re[:, g0:g0 + gn, :])
                    st = io.tile([128, cfg.TBGRP, W3], F16, tag="tstage")
                    for j in range(gn):
                        t = g0 + j
                        xT_ps = ps.tile([128, 128], F32, tag="xT")
                        nc.tensor.transpose(xT_ps[:], xs[:, j, :], ident[:])
                        xT16 = wk.tile([128, 128], F16, tag="xT16")
                        nc.vector.tensor_copy(xT16[:], xT_ps[:])
                        kqv_ps = ps.tile([128, W3], F32, tag="kqv")
                        nc.tensor.matmul(kqv_ps[:], lhsT=xT16[:], rhs=wn16[:],
                                         start=True, stop=False)
                        nc.tensor.matmul(kqv_ps[:], lhsT=ones_r[:], rhs=bn16[:],
                                         start=False, stop=True)
                        bt = wk.tile([128, HID], F32, tag="btmp")
                        nc.vector.tensor_tensor(bt[:], kqv_ps[:, 0:HID],
                                                bqe_t[:], op=OP.mult)
                        bs = wk.tile([128, H], F32, tag="bsum")
                        nc.vector.tensor_reduce(
                            bs[:], bt[:].rearrange("p (h d) -> p h d", h=H),
                            axis=mybir.AxisListType.X, op=OP.add)
                        nc.scalar.activation(bexp[:, t, :], bs[:], AF.Exp,
                                             scale=cfg.SCALE)
                        nc.vector.tensor_copy(st[:, j, 0:KQ], kqv_ps[:, 0:KQ])
                        nc.vector.tensor_tensor(
                            st[:, j, KQ:W3].rearrange("p (h d) -> p h d", h=H),
                            kqv_ps[:, KQ:W3].rearrange("p (h d) -> p h d", h=H),
                            bexp[:, t, :].to_broadcast([128, H, D]),
                            op=OP.mult)
                    tab_writes.append(nc.sync.dma_start(
                        out=nkq_re[:, g0:g0 + gn, :], in_=st[:, :gn, 0:KQ]))
                    tab_writes.append(nc.sync.dma_start(
                        out=nv_re[:, g0:g0 + gn, :], in_=st[:, :gn, KQ:W3]))

            # ---- phase 2: endpoint loop ----
            sched = []
            for t in range(NT):
                for j in range(kt[t]):
                    sched.append((t, j == 0, j == kt[t] - 1))
            assert len(sched) == nchunk

            with (
                tc.tile_pool(name="io2", bufs=2) as io,
                tc.tile_pool(name="wk2", bufs=3) as wk,
                tc.tile_pool(name="ps2w", bufs=2, space="PSUM") as psw,
                tc.tile_pool(name="ps2e", bufs=4, space="PSUM") as pse,
                tc.tile_pool(name="ps2x", bufs=2, space="PSUM") as psx,
            ):
                # fence: one op depending on all table writes; reads dep on it
                fence_t = wk.tile([1, 4], F32, tag="fence")
                fence = nc.vector.memset(fence_t[:], 0.0)
                for wi in tab_writes:
                    add_dep_helper(fence.ins, wi.ins,
                                   reason="fence after table writes")
                eabuf = win_ps = wbuf = lrbuf = None
                prod = contrib = onehot = None
                eq_group = []
                cur_tw = -1
                for k in range(nchunk):
                    t_k = sched[k][0]
                    if t_k // 8 != cur_tw:
                        cur_tw = t_k // 8
                        tn = min(8, NT - cur_tw * 8)
                        wbuf = io.tile([128, 8, KQ], F16, tag="wbuf")
                        w_inst = nc.sync.dma_start(
                            out=wbuf[:, :tn, :],
                            in_=nkq_re[:, cur_tw * 8:cur_tw * 8 + tn, :])
                        add_dep_helper(w_inst.ins, fence.ins,
                                       reason="window tile reads node table")
                    if k % 16 == 0:
                        h2 = k // 16
                        lw = min(16, nchunk - h2 * 16) * 128
                        lrbuf = io.tile([128, 16 * 128], F16, tag="lrbuf")
                        nc.sync.dma_start(
                            out=lrbuf[:, :lw],
                            in_=lidrep_in[:, h2 * 2048:h2 * 2048 + lw])
                    # transposed one-hot (window-node-major) -> PE expansion
                    onehotT = wk.tile([128, 128], F16, tag="onehotT")
                    nc.vector.tensor_tensor(
                        onehotT[:],
                        iota_pc[:].to_broadcast([128, 128]),
                        lrbuf[:, (k % 16) * 128:(k % 16 + 1) * 128],
                        op=OP.is_equal)
                    expkq_ps = psx.tile([128, KQ], F32, tag="expkq")
                    nc.tensor.matmul(expkq_ps[:], lhsT=onehotT[:],
                                     rhs=wbuf[:, t_k % 8, :],
                                     start=True, stop=True)
                    gbuf = wk.tile([128, KQ], F16, tag="gbuf")
                    nc.vector.tensor_copy(gbuf[:], expkq_ps[:])
                    if k % cfg.EABUF == 0:
                        h = k // cfg.EABUF
                        w = cfg.EABUF * 64
                        eabuf = io.tile([128, w], F16, tag="eabuf")
                        nc.gpsimd.dma_start(
                            out=eabuf[:], in_=ea_pT[:, h * w:(h + 1) * w])
                    if k % 4 == 0:
                        j4 = k // 4
                        prod = wk.tile([128, 4, KQ], F16, tag="prod")
                        contrib = wk.tile([128, 4, 8 + HID], F16, tag="contrib")
                        onehot = wk.tile([128, 4, 128], F16, tag="onehot")
                        eq_group = []
                        nc.vector.tensor_tensor(
                            onehot[:],
                            lid_sb[:, j4 * 4:(j4 + 1) * 4].to_broadcast(
                                [128, 4, 128]),
                            iota4[:], op=OP.is_equal)
                    if cfg.dbg and k == 0:
                        nc.sync.dma_start(out=dbg_gb[:], in_=gbuf[:])
                    q = k % 4
                    pr = (k // 2) % (cfg.EABUF // 2)
                    half = (k % 2) * ED
                    eqkv = pse.tile([128, W3], F32, tag="eqkv")
                    eq_group.append(eqkv)
                    nc.tensor.matmul(
                        eqkv[:],
                        lhsT=eabuf[half:half + ED, pr * 128:(pr + 1) * 128],
                        rhs=we_dup[half:half + ED, :], start=True, stop=True)
                    nc.vector.tensor_tensor(prod[:, q, :], eqkv[:, 0:KQ],
                                            gbuf[:], op=OP.mult)
                    if cfg.dbg and k < 4:
                        if k == 0:
                            dbg_eqs_t = pp.tile([128, 4, W3], F32,
                                                tag="dbgeqs")
                        nc.vector.tensor_copy(dbg_eqs_t[:, k, :], eqkv[:])
                        if k == 3:
                            nc.sync.dma_start(out=dbg_eq[:], in_=dbg_eqs_t[:])
                    if q == 3:
                        sraw = wk.tile([128, 4, 2 * H], F32, tag="sraw")
                        nc.vector.tensor_reduce(
                            sraw[:],
                            prod[:].rearrange("p c (g d) -> p c g d", d=D),
                            axis=mybir.AxisListType.X, op=OP.add)
                        nc.scalar.activation(contrib[:, :, 0:2 * H], sraw[:],
                                             AF.Exp, scale=cfg.SCALE)
                        for q2 in range(4):
                            k2 = k - 3 + q2
                            eq2 = eq_group[q2]
                            nc.vector.tensor_tensor(
                                contrib[:, q2, 8:].rearrange(
                                    "p (h d) -> p h d", h=H),
                                contrib[:, q2, H:2 * H].to_broadcast(
                                    [128, H, D]),
                                eq2[:, KQ:W3].rearrange("p (h d) -> p h d",
                                                        h=H),
                                op=OP.mult)
                            t2, wf2, wl2 = sched[k2]
                            if wf2:
                                win_ps = psw.tile([ACC_C, 128], F32, tag="win")
                            nc.tensor.matmul(win_ps[:],
                                             lhsT=contrib[:, q2, :],
                                             rhs=onehot[:, q2, :],
                                             start=wf2, stop=wl2)
                            if wl2:
                                nc.vector.tensor_copy(
                                    accum[:, t2 * 128:(t2 + 1) * 128],
                                    win_ps[:])
                        if cfg.dbg and k == 3:
                            nc.sync.dma_start(out=dbg_ct[:], in_=contrib[:])
                            nc.sync.dma_start(out=dbg_oh[:], in_=onehot[:])

            if cfg.dbg:
                nc.sync.dma_start(out=dbg_acc[:], in_=accum[:])
                nc.sync.dma_start(out=dbg_tab[:], in_=nkq_tab[:])
                nc.sync.dma_start(out=dbg_nv[:], in_=nv_tab[:])

            # ---- phase 3: AllReduce ----
            nc.gpsimd.dma_start(b_in[:], accum[:])
            if cfg.NCORE > 1:
                nc.gpsimd.collective_compute(
                    "AllReduce", OP.add,
                    replica_groups=[list(range(cfg.NCORE))],
                    ins=[b_in.opt()], outs=[b_out.opt()])
            else:
                nc.gpsimd.dma_start(b_out[:], b_in[:])

            if cfg.dbg:
                dbg_bt_sb = pp.tile([ACC_C, NPAD], F32, tag="dbgbt")
                nc.sync.dma_start(out=dbg_bt_sb[:], in_=b_out[:])
                nc.sync.dma_start(out=dbg_bout[:], in_=dbg_bt_sb[:])

            # ---- phase 4a: pooled vector -> edge row ----
            with (
                tc.tile_pool(name="io4", bufs=2) as io,
                tc.tile_pool(name="wk4", bufs=2) as wk,
                tc.tile_pool(name="ps4", bufs=1, space="PSUM") as ps,
                tc.tile_pool(name="ps4w", bufs=1, space="PSUM") as psw,
                tc.tile_pool(name="ps4t", bufs=2, space="PSUM") as pst,
            ):
                nv_all = io.tile([128, NT, HID + 1], F16, tag="nvall", bufs=1)
                nc.vector.memset(nv_all[:], 1.0)
                nv_rd = nc.sync.dma_start(out=nv_all[:, :, 0:HID],
                                          in_=nv_re[:, :, :])
                for wi in tab_writes:
                    add_dep_helper(nv_rd.ins, wi.ins,
                                   reason="nv readback after table write")
                pv_ps = ps.tile([HID + 1, H], F32, tag="pv")
                nblk = cfg.W1BLK
                for b0 in range(0, NT, nblk):
                    bn_ = min(nblk, NT - b0)
                    w1blk = io.tile([H, nblk * 128], F32, tag="w1blk")
                    nc.sync.dma_start(
                        out=w1blk[:, :bn_ * 128],
                        in_=b_out[0:H, b0 * 128:(b0 + bn_) * 128])
                    for j in range(bn_):
                        t = b0 + j
                        w1T_ps = pst.tile([128, H], F32, tag="w1T")
                        nc.tensor.transpose(
                            w1T_ps[:], w1blk[:, j * 128:(j + 1) * 128],
                            ident[0:H, 0:H])
                        w1t16 = wk.tile([128, H], F16, tag="w1t16")
                        nc.vector.tensor_copy(w1t16[:], w1T_ps[:])
                        nc.tensor.matmul(pv_ps[:, :],
                                         lhsT=nv_all[:, t, :],
                                         rhs=w1t16[:], start=(t == 0),
                                         stop=(t == NT - 1))
                t1_16 = wk.tile([HID, H], F16, tag="t1")
                nc.vector.tensor_tensor(t1_16[:], pv_ps[0:HID, :], mmat16[:],
                                        op=OP.mult)
                den1_sb = wk.tile([1, H], F32, tag="den1sb")
                nc.vector.tensor_copy(den1_sb[:], pv_ps[HID:HID + 1, 0:H])
                den1T_ps = psw.tile([H, 1], F32, tag="den1T")
                nc.tensor.transpose(den1T_ps[:], den1_sb[:], ident[0:1, 0:1])
                den1r = wk.tile([H, 1], F32, tag="den1r")
                nc.vector.reciprocal(den1r[:], den1T_ps[:])
                wpv_ps = psw.tile([H, ED], F32, tag="wpv")
                nc.tensor.matmul(wpv_ps[:], lhsT=t1_16[:], rhs=woe16[:],
                                 start=True, stop=True)
                scl16 = wk.tile([H, ED], F16, tag="scl")
                nc.vector.tensor_scalar(scl16[:], wpv_ps[:], den1r[:], None,
                                        op0=OP.mult)
                row_ps = psw.tile([1, ED], F32, tag="rowps")
                nc.tensor.matmul(row_ps[:], lhsT=ones_4[:], rhs=scl16[:],
                                 start=True, stop=True)
                row16 = wk.tile([1, ED], F16, tag="row16")
                nc.vector.tensor_tensor(row16[:], row_ps[:], boe_sb[:],
                                        op=OP.add)
                rowb_ps = psw.tile([128, ED], F32, tag="rowb")
                nc.tensor.matmul(rowb_ps[:], lhsT=ones_r[:], rhs=row16[:],
                                 start=True, stop=True)
                rowb = cp.tile([128, 1, ED], F32)
                nc.vector.tensor_copy(rowb[:, 0, :], rowb_ps[:])

            # ---- phase 4b: edge output ----
            with (
                tc.tile_pool(name="io5", bufs=3) as io,
            ):
                for s0 in range(0, ET, cfg.ESUB):
                    sn = min(cfg.ESUB, ET - s0)
                    et = io.tile([128, cfg.ESUB, ED], F32, tag="etile")
                    nc.sync.dma_start(out=et[:, :sn, :],
                                      in_=ea_sh[:, s0:s0 + sn, :])
                    nc.vector.tensor_tensor(
                        et[:, :sn, :], et[:, :sn, :],
                        rowb[:].to_broadcast([128, sn, ED]), op=OP.add)
                    nc.sync.dma_start(out=out_e[:, s0:s0 + sn, :],
                                      in_=et[:, :sn, :])

            # ---- phase 4c: node output (redundant on every core) ----
            with (
                tc.tile_pool(name="wk6", bufs=3) as wk,
                tc.tile_pool(name="ps6", bufs=2, space="PSUM") as ps,
            ):
                for t in range(NT):
                    base = t * 128
                    rows = min(128, cfg.N - base)
                    if rows <= 0:
                        break
                    den2 = wk.tile([H, 128], F32, tag="den2")
                    aggv = wk.tile([HID, 128], F32, tag="aggv")
                    nc.sync.dma_start(out=den2[:, :rows],
                                      in_=b_out[H:2 * H, base:base + rows])
                    nc.sync.dma_start(out=aggv[:, :rows],
                                      in_=b_out[2 * H:ACC_C, base:base + rows])
                    den2_16 = wk.tile([H, 128], F16, tag="den216")
                    nc.vector.tensor_copy(den2_16[:, :rows], den2[:, :rows])
                    dx_ps = ps.tile([HID, 128], F32, tag="dxps")
                    nc.tensor.matmul(dx_ps[:, :rows], lhsT=emat16[:],
                                     rhs=den2_16[:, :rows], start=True,
                                     stop=True)
                    at = wk.tile([HID, 128], F32, tag="at")
                    nc.vector.scalar_tensor_tensor(
                        at[:, :rows], dx_ps[:, :rows], bve_sb[:],
                        aggv[:, :rows], op0=OP.mult, op1=OP.add)
                    dcl = wk.tile([HID, 128], F32, tag="dcl")
                    nc.vector.tensor_scalar(dcl[:, :rows], dx_ps[:, :rows],
                                            1e-20, None, op0=OP.max)
                    drec = wk.tile([HID, 128], F32, tag="drec")
                    nc.vector.reciprocal(drec[:, :rows], dcl[:, :rows])
                    agg16 = wk.tile([HID, 128], F16, tag="agg16")
                    nc.vector.tensor_tensor(agg16[:, :rows], at[:, :rows],
                                            drec[:, :rows], op=OP.mult)
                    o_ps = ps.tile([128, ND], F32, tag="ops")
                    nc.tensor.matmul(o_ps[:rows, :], lhsT=agg16[:, :rows],
                                     rhs=won16[:], start=True, stop=False)
                    nc.tensor.matmul(o_ps[:rows, :], lhsT=ones_r[:, :rows],
                                     rhs=bon16[:], start=False, stop=True)
                    xres = wk.tile([128, ND], F32, tag="xres")
                    nc.sync.dma_start(out=xres[:rows, :],
                                      in_=x_in[base:base + rows, :])
                    ot = wk.tile([128, ND], F32, tag="ot")
                    nc.vector.tensor_add(ot[:rows, :], o_ps[:rows, :],
                                         xres[:rows, :])
                    nc.sync.dma_start(out=out_x[base:base + rows, :],
                                      in_=ot[:rows, :])

    nc.compile()
    return nc


# --------------------------------------------------------------------------
# entry point
# --------------------------------------------------------------------------

_CACHE = {}


def run(cfg, inputs, run_fn=None):
    x = np.asarray(inputs["x"], np.float32)
    edge_attr = np.asarray(inputs["edge_attr"], np.float32)
    edge_index = np.asarray(inputs["edge_index"])

    meta, per_core = _prep(cfg, edge_index, edge_attr)
    key = (cfg, tuple(meta["kt"]), meta["nchunk"])
    if key not in _CACHE:
        _CACHE[key] = _build(cfg, meta)
    nc = _CACHE[key]

    x_pad = np.zeros((cfg.NPAD, cfg.ND), np.float32)
    x_pad[:cfg.N] = x
    H, D, HID = cfg.H, cfg.D, cfg.HID
    wn_cat = np.concatenate(
        [inputs["Wkn"], inputs["Wqn"], inputs["Wvn"]], axis=1)
    bn_cat = np.concatenate(
        [inputs["bkn"], inputs["bqn"], inputs["bvn"]])[None, :]
    we_cat = np.concatenate(
        [inputs["Wqe"], inputs["Wke"], inputs["Wve"]], axis=1)
    emat = np.zeros((H, HID), np.float32)
    for h in range(H):
        emat[h, h * D:(h + 1) * D] = 1.0
    shared = {
        "x": x_pad,
        "Wn_cat": np.ascontiguousarray(wn_cat, dtype=np.float32),
        "bn_cat": np.ascontiguousarray(bn_cat, dtype=np.float32),
        "We_cat": np.ascontiguousarray(we_cat, dtype=np.float32),
        "Won": np.asarray(inputs["Won"], np.float32),
        "bon": np.asarray(inputs["bon"], np.float32)[None, :].copy(),
        "Woe": np.asarray(inputs["Woe"], np.float32),
        "boe": np.asarray(inputs["boe"], np.float32)[None, :].copy(),
        "bqe": np.asarray(inputs["bqe"], np.float32)[None, :].copy(),
        "bve_col": np.asarray(inputs["bve"], np.float32)[:, None].copy(),
        "E_mat": emat, "M_mat": np.ascontiguousarray(emat.T),
    }
    in_maps = []
    for c in range(cfg.NCORE):
        m = dict(shared)
        m.update(per_core[c])
        m["ea_shard"] = _pack_edge_shard(cfg, edge_attr, c)
        in_maps.append(m)

    if run_fn is not None:
        results = run_fn(nc, in_maps)
    else:
        res = run_bass_kernel_spmd(nc, in_maps,
                                   core_ids=list(range(cfg.NCORE)))
        results = res.results

    upd_x = np.asarray(results[0]["out_x"], np.float32)
    upd_e = np.zeros((cfg.E, cfg.ED), np.float32)
    for c in range(cfg.NCORE):
        pe = results[c]["out_e"].transpose(1, 0, 2).reshape(cfg.EPAD, cfg.ED)
        upd_e[c * cfg.EC:(c + 1) * cfg.EC] = pe[:cfg.EC]
    return upd_x, upd_e


def kernel(**inputs):
    return run(CFG_FULL, inputs)
